# revision 6
# baseline (speedup 1.0000x reference)
"""Trainium2 Bass kernel for nn_ADTNSublayer: permuted block-diagonal linear.

y[t, g*GO:(g+1)*GO] = W[g] @ x[t, perm[g*GS:(g+1)*GS]] + b[g]

Strategy: data-parallel over the 16384 tokens across 8 NeuronCores (2048
tokens/core, no collectives). Each core receives its x-shard feature-major
and pre-cast to bf16 (xT [4096, 2048] bf16). The pipeline is split over
FEATURES, not tokens: each of 8 stages gathers 512 permuted feature rows
(= 4 dest blocks) across the full 2048-token range in one dma_gather of
512 descriptors x 4 KiB (full rows), computes the 4 blocks' bf16 matmuls
(f32 PSUM accumulation, bias fused into the PSUM->SBUF evacuation on
ScalarE/VectorE alternating), and stores 512 output rows as one DMA of
512 x 4 KiB descriptors. Full-row descriptors keep both the read and the
write stream at maximum per-descriptor payload (4 KiB) with only 4096
descriptors each way per exec.

Precision: x, W and y are bf16 on device (f32 accumulation in PSUM). The
per-element error is the bf16 quantization of the inputs carried through a
128-term dot product (~2e-3 of a typical |y|) plus the bf16 output
quantization (~2e-3 max rel), far inside the 2e-2 gate. Traffic per core is
16 MiB x read + 16 MiB y write = 32 MiB (vs 48 MiB for the f32-read
variant), which puts the DMA roofline at ~82 us/core at the ~408 GB/s
per-stack HBM3 peak.

The host only does layout transforms (sharding, transposes, f32->bf16
casts, index retyping) - the permutation gather, the matmuls and the bias
add all execute on device.
"""

import sys

import numpy as np

try:
    import concourse.bass as bass  # noqa: F401
except ImportError:  # pragma: no cover - fresh-dir fallback
    sys.path.insert(0, "/opt/trn_rl_repo")

import concourse.bacc as bacc
import concourse.mybir as mybir
import concourse.tile as tile

F32 = mybir.dt.float32
BF16 = mybir.dt.bfloat16
I16 = mybir.dt.int16
Identity = mybir.ActivationFunctionType.Identity

B, S, F = 4, 4096, 4096
G, GS, GO = 32, 128, 128
N_CORES = 8
TOK = B * S                    # 16384 tokens
TPC = TOK // N_CORES           # 2048 tokens per core
FS = 512                       # feature rows gathered/stored per stage
NF = F // FS                   # 8 pipeline stages
CB = FS // GS                  # 4 g-blocks per stage
TW = 512                       # tokens per matmul (PSUM free-dim limit)
NTW = TPC // TW                # 4 matmul windows per block


def build_nc(reps: int = 1):
    """Build the per-core Bass graph. `reps` repeats the whole compute body
    (same data) for benchmarking; kernel() uses reps=1."""
    nc = bacc.Bacc(None, num_swdge_queues=2)
    xT = nc.declare_dram_parameter("xT", [F, TPC], BF16, isOutput=False)
    wT = nc.declare_dram_parameter("wT", [GS, G * GO], BF16, isOutput=False)
    bT = nc.declare_dram_parameter("bT", [GO, G], F32, isOutput=False)
    idx = nc.declare_dram_parameter("idx", [128, F // 16], I16, isOutput=False)
    yT = nc.declare_dram_parameter("yT", [F, TPC], BF16, isOutput=True)

    with tile.TileContext(nc) as tc:
        with (
            tc.tile_pool(name="const", bufs=1) as cpool,
            tc.tile_pool(name="gather", bufs=3) as gpool,
            tc.tile_pool(name="out", bufs=3) as opool,
            tc.tile_pool(name="psum", bufs=8, space="PSUM") as ppool,
        ):
            w_t = cpool.tile([GS, G * GO], BF16)
            b_t = cpool.tile([GO, G], F32)
            idx_t = cpool.tile([128, F // 16], I16)
            # idx first and on the ACT HWDGE ring: the first gather's SWDGE
            # descriptor generation only needs idx, so it overlaps the W load
            # instead of queueing behind it.
            nc.scalar.dma_start(idx_t[:], idx[:])
            nc.sync.dma_start(w_t[:], wT[:])
            nc.scalar.dma_start(b_t[:], bT[:])

            ic = FS // 16                       # idx columns per stage
            for _ in range(reps):
                for fs in range(NF):
                    g_t = gpool.tile([128, CB, TPC], BF16, tag="gather")
                    nc.gpsimd.dma_gather(
                        g_t[:],
                        xT[:],
                        idx_t[:, fs * ic:(fs + 1) * ic],
                        num_idxs=FS,
                        num_idxs_reg=FS,
                        elem_size=TPC,
                        elem_step=TPC,
                        single_packet=False,
                        queue_num=fs % 2,
                    )
                    o_t = opool.tile([128, CB, TPC], BF16, tag="out")
                    for c in range(CB):
                        g = fs * CB + c
                        for tw in range(NTW):
                            ps = ppool.tile([GO, TW], F32, tag="ps")
                            nc.tensor.matmul(
                                ps[:],
                                w_t[:, g * GO:(g + 1) * GO],
                                g_t[:, c, tw * TW:(tw + 1) * TW],
                                start=True,
                                stop=True,
                            )
                            dst = o_t[:, c, tw * TW:(tw + 1) * TW]
                            if (c * NTW + tw) % 2 == 0:
                                nc.scalar.activation(
                                    dst, ps[:], Identity,
                                    bias=b_t[:, g:g + 1],
                                )
                            else:
                                nc.vector.tensor_scalar_add(
                                    dst, ps[:], b_t[:, g:g + 1],
                                )
                    ydst = yT[fs * FS:(fs + 1) * FS, :]
                    ydst = ydst.rearrange("(c p) t -> p c t", p=128)
                    # alternate store rings (SYNC / ACT HWDGE) so two DMA
                    # engines drain the write stream in parallel.
                    eng = nc.sync if fs % 2 == 0 else nc.scalar
                    eng.dma_start(ydst, o_t[:])
    nc.compile()
    return nc


def make_runner(nc, n_cores=N_CORES):
    """Compile nc into a reusable jitted SPMD callable.

    Returns (run_fn, out_names): run_fn(in_maps) -> list of per-core output
    dicts. The jit/NEFF compile happens once; later calls only upload inputs.
    """
    import jax
    from jax.sharding import Mesh, PartitionSpec, NamedSharding
    from jax.experimental.shard_map import shard_map
    from concourse import bass2jax

    bass2jax.install_neuronx_cc_hook()

    in_names, out_names, out_avals, zero_outs = [], [], [], []
    for alloc in nc.m.functions[0].allocations:
        if not isinstance(alloc, mybir.MemoryLocationSet):
            continue
        name = alloc.memorylocations[0].name
        if alloc.kind == "ExternalInput":
            in_names.append(name)
        elif alloc.kind == "ExternalOutput":
            shape = tuple(alloc.tensor_shape)
            dtype = mybir.dt.np(alloc.dtype)
            out_names.append(name)
            out_avals.append(jax.core.ShapedArray(shape, dtype))
            zero_outs.append(np.zeros(shape, dtype))
    partition_name = (
        nc.partition_id_tensor.name if nc.partition_id_tensor else None
    )
    if partition_name is not None and partition_name in in_names:
        in_names.remove(partition_name)
    n_params = len(in_names)
    all_in_names = list(in_names) + list(out_names)
    if partition_name is not None:
        all_in_names = all_in_names + [partition_name]

    def _body(*args):
        operands = list(args)
        if partition_name is not None:
            operands.append(bass2jax.partition_id_tensor())
        outs = bass2jax._bass_exec_p.bind(
            *operands,
            out_avals=tuple(out_avals),
            in_names=tuple(all_in_names),
            out_names=tuple(out_names),
            lowering_input_output_aliases=(),
            sim_require_finite=True,
            sim_require_nnan=True,
            nc=nc,
        )
        return tuple(outs)

    devices = jax.devices()[:n_cores]
    assert len(devices) == n_cores, (
        f"need {n_cores} neuron cores, have {len(jax.devices())}"
    )
    mesh = Mesh(np.asarray(devices), ("core",))
    spec = PartitionSpec("core")
    fn = jax.jit(
        shard_map(
            _body,
            mesh=mesh,
            in_specs=(spec,) * (n_params + len(out_names)),
            out_specs=(spec,) * len(out_names),
            check_rep=False,
        ),
        keep_unused=True,
    )
    sharding = NamedSharding(mesh, spec)
    zero_args = [
        jax.device_put(np.concatenate([z] * n_cores, axis=0), sharding)
        for z in zero_outs
    ]

    def run(in_maps, device_args=None):
        if device_args is None:
            device_args = [
                jax.device_put(
                    np.concatenate(
                        [np.asarray(m[name]) for m in in_maps], axis=0
                    ),
                    sharding,
                )
                for name in in_names
            ]
        outs = fn(*device_args, *zero_args)
        jax.block_until_ready(outs)
        res = []
        for c in range(n_cores):
            d = {}
            for i, name in enumerate(out_names):
                arr = np.asarray(outs[i])
                per = arr.shape[0] // n_cores
                d[name] = arr[c * per:(c + 1) * per]
            res.append(d)
        return res

    run.in_names = in_names
    run.sharding = sharding
    run.fn = fn
    run.zero_args = zero_args
    return run, out_names


def make_in_maps(x, input_perm, W, b):
    """Host-side sharding / layout transforms -> per-core input dicts."""
    bf16 = mybir.dt.np(BF16)
    toks = np.asarray(x, dtype=np.float32).reshape(TOK, F)
    wT = np.ascontiguousarray(
        np.transpose(np.asarray(W, dtype=np.float32), (2, 0, 1))
    ).reshape(GS, G * GO).astype(bf16)
    bT = np.ascontiguousarray(np.asarray(b, dtype=np.float32).T)
    perm = np.asarray(input_perm).astype(np.int64)
    # idx table, stage-blocked: stage fs occupies columns [fs*FS/16,
    # (fs+1)*FS/16); within a stage, local index j sits at row j%16,
    # column j//16 (the gather's 16-partition wrap), replicated x8.
    p16 = perm.astype(np.int16).reshape(NF, FS // 16, 16)
    idx_w = np.concatenate([s.T for s in p16], axis=1)        # [16, F//16]
    idx_full = np.ascontiguousarray(np.tile(idx_w, (8, 1)))   # [128, F//16]

    in_maps = []
    for c in range(N_CORES):
        shard = toks[c * TPC:(c + 1) * TPC]                   # [TPC, F]
        xT = np.ascontiguousarray(shard.T).astype(bf16)       # [F, TPC] bf16
        in_maps.append({"xT": xT, "wT": wT, "bT": bT, "idx": idx_full})
    return in_maps


def assemble_output(results, dtype):
    """Per-core yT [F, TPC] -> full y [B, S, F]."""
    parts = [
        np.ascontiguousarray(results[c]["yT"].astype(np.float32).T)
        for c in range(N_CORES)
    ]
    y = np.concatenate(parts, axis=0).reshape(B, S, F)
    return y.astype(dtype, copy=False)


_RUNNER_CACHE = {}


def _get_runner():
    if "run" not in _RUNNER_CACHE:
        nc = build_nc(reps=1)
        run, out_names = make_runner(nc)
        _RUNNER_CACHE["run"] = run
    return _RUNNER_CACHE["run"]


def kernel(**inputs) -> np.ndarray:
    x = inputs["x"]
    run = _get_runner()
    in_maps = make_in_maps(x, inputs["input_perm"], inputs["W"], inputs["b"])
    results = run(in_maps)
    return assemble_output(results, np.asarray(x).dtype)


if __name__ == "__main__":
    rng = np.random.default_rng(0)
    x = rng.standard_normal((B, S, F), dtype=np.float32)
    perm = rng.permutation(F).astype(np.int64)
    W = (rng.standard_normal((G, GO, GS), dtype=np.float32) / np.sqrt(GS))
    b = rng.standard_normal((G, GO), dtype=np.float32) * 0.01
    y = kernel(x=x, input_perm=perm, W=W, b=b)
    ref = np.einsum("bsgi,goi->bsgo",
                    x[..., perm].reshape(B, S, G, GS), W) + b
    ref = ref.reshape(B, S, F)
    err = np.abs(y - ref).max() / np.abs(ref).max()
    print("self-check rel err:", err)


# revision 9
# speedup vs baseline: 1.0575x; 1.0575x over previous
"""Trainium2 Bass kernel for nn_ADTNSublayer: permuted block-diagonal linear.

y[t, g*GO:(g+1)*GO] = W[g] @ x[t, perm[g*GS:(g+1)*GS]] + b[g]

Strategy: data-parallel over the 16384 tokens across 8 NeuronCores (2048
tokens/core, no collectives). Each core receives its x-shard feature-major
and pre-cast to bf16 (xT [4096, 2048] bf16). The pipeline is split over
FEATURES, not tokens: each of 8 stages gathers 512 permuted feature rows
(= 4 dest blocks) across the full 2048-token range in one dma_gather of
512 descriptors x 4 KiB (full rows), computes the 4 blocks' bf16 matmuls
(f32 PSUM accumulation, bias fused into the PSUM->SBUF evacuation on
ScalarE/VectorE alternating), and stores 512 output rows as one DMA of
512 x 4 KiB descriptors. Full-row descriptors keep both the read and the
write stream at maximum per-descriptor payload (4 KiB) with only 4096
descriptors each way per exec.

Precision: x, W and y are bf16 on device (f32 accumulation in PSUM). The
per-element error is the bf16 quantization of the inputs carried through a
128-term dot product (~2e-3 of a typical |y|) plus the bf16 output
quantization (~2e-3 max rel), far inside the 2e-2 gate. Traffic per core is
16 MiB x read + 16 MiB y write = 32 MiB (vs 48 MiB for the f32-read
variant), which puts the DMA roofline at ~82 us/core at the ~408 GB/s
per-stack HBM3 peak.

The host only does layout transforms (sharding, transposes, f32->bf16
casts, index retyping) - the permutation gather, the matmuls and the bias
add all execute on device.
"""

import sys

import numpy as np

try:
    import concourse.bass as bass  # noqa: F401
except ImportError:  # pragma: no cover - fresh-dir fallback
    sys.path.insert(0, "/opt/trn_rl_repo")

import concourse.bacc as bacc
import concourse.mybir as mybir
import concourse.tile as tile

F32 = mybir.dt.float32
BF16 = mybir.dt.bfloat16
I16 = mybir.dt.int16
Identity = mybir.ActivationFunctionType.Identity

B, S, F = 4, 4096, 4096
G, GS, GO = 32, 128, 128
N_CORES = 8
TOK = B * S                    # 16384 tokens
TPC = TOK // N_CORES           # 2048 tokens per core
FS = 512                       # feature rows gathered/stored per stage
NF = F // FS                   # 8 pipeline stages
CB = FS // GS                  # 4 g-blocks per stage
TW = 512                       # tokens per matmul (PSUM free-dim limit)
NTW = TPC // TW                # 4 matmul windows per block


def build_nc(reps: int = 1):
    """Build the per-core Bass graph. `reps` repeats the whole compute body
    (same data) for benchmarking; kernel() uses reps=1."""
    nc = bacc.Bacc(None)
    xT = nc.declare_dram_parameter("xT", [F, TPC], BF16, isOutput=False)
    wT = nc.declare_dram_parameter("wT", [GS, G * GO], BF16, isOutput=False)
    bT = nc.declare_dram_parameter("bT", [GO, G], F32, isOutput=False)
    idx = nc.declare_dram_parameter("idx", [128, F // 16], I16, isOutput=False)
    yT = nc.declare_dram_parameter("yT", [F, TPC], BF16, isOutput=True)

    with tile.TileContext(nc) as tc:
        with (
            tc.tile_pool(name="const", bufs=1) as cpool,
            tc.tile_pool(name="gather", bufs=3) as gpool,
            tc.tile_pool(name="out", bufs=3) as opool,
            tc.tile_pool(name="psum", bufs=8, space="PSUM") as ppool,
        ):
            w_t = cpool.tile([GS, G * GO], BF16)
            b_t = cpool.tile([GO, G], F32)
            idx_t = cpool.tile([128, F // 16], I16)
            # idx first and on the ACT HWDGE ring: the first gather's SWDGE
            # descriptor generation only needs idx, so it overlaps the W load
            # instead of queueing behind it.
            nc.scalar.dma_start(idx_t[:], idx[:])
            nc.sync.dma_start(w_t[:], wT[:])
            nc.scalar.dma_start(b_t[:], bT[:])

            ic = FS // 16                       # idx columns per stage
            for _ in range(reps):
                for fs in range(NF):
                    g_t = gpool.tile([128, CB, TPC], BF16, tag="gather")
                    nc.gpsimd.dma_gather(
                        g_t[:],
                        xT[:],
                        idx_t[:, fs * ic:(fs + 1) * ic],
                        num_idxs=FS,
                        num_idxs_reg=FS,
                        elem_size=TPC,
                        elem_step=TPC,
                        single_packet=False,
                    )
                    o_t = opool.tile([128, CB, TPC], BF16, tag="out")
                    for c in range(CB):
                        g = fs * CB + c
                        for tw in range(NTW):
                            ps = ppool.tile([GO, TW], F32, tag="ps")
                            nc.tensor.matmul(
                                ps[:],
                                w_t[:, g * GO:(g + 1) * GO],
                                g_t[:, c, tw * TW:(tw + 1) * TW],
                                start=True,
                                stop=True,
                            )
                            dst = o_t[:, c, tw * TW:(tw + 1) * TW]
                            if (c * NTW + tw) % 2 == 0:
                                nc.scalar.activation(
                                    dst, ps[:], Identity,
                                    bias=b_t[:, g:g + 1],
                                )
                            else:
                                nc.vector.tensor_scalar_add(
                                    dst, ps[:], b_t[:, g:g + 1],
                                )
                    ydst = yT[fs * FS:(fs + 1) * FS, :]
                    ydst = ydst.rearrange("(c p) t -> p c t", p=128)
                    nc.sync.dma_start(ydst, o_t[:])
    nc.compile()
    return nc


def make_runner(nc, n_cores=N_CORES):
    """Compile nc into a reusable jitted SPMD callable.

    Returns (run_fn, out_names): run_fn(in_maps) -> list of per-core output
    dicts. The jit/NEFF compile happens once; later calls only upload inputs.
    """
    import jax
    from jax.sharding import Mesh, PartitionSpec, NamedSharding
    from jax.experimental.shard_map import shard_map
    from concourse import bass2jax

    bass2jax.install_neuronx_cc_hook()

    in_names, out_names, out_avals, zero_outs = [], [], [], []
    for alloc in nc.m.functions[0].allocations:
        if not isinstance(alloc, mybir.MemoryLocationSet):
            continue
        name = alloc.memorylocations[0].name
        if alloc.kind == "ExternalInput":
            in_names.append(name)
        elif alloc.kind == "ExternalOutput":
            shape = tuple(alloc.tensor_shape)
            dtype = mybir.dt.np(alloc.dtype)
            out_names.append(name)
            out_avals.append(jax.core.ShapedArray(shape, dtype))
            zero_outs.append(np.zeros(shape, dtype))
    partition_name = (
        nc.partition_id_tensor.name if nc.partition_id_tensor else None
    )
    if partition_name is not None and partition_name in in_names:
        in_names.remove(partition_name)
    n_params = len(in_names)
    all_in_names = list(in_names) + list(out_names)
    if partition_name is not None:
        all_in_names = all_in_names + [partition_name]

    def _body(*args):
        operands = list(args)
        if partition_name is not None:
            operands.append(bass2jax.partition_id_tensor())
        outs = bass2jax._bass_exec_p.bind(
            *operands,
            out_avals=tuple(out_avals),
            in_names=tuple(all_in_names),
            out_names=tuple(out_names),
            lowering_input_output_aliases=(),
            sim_require_finite=True,
            sim_require_nnan=True,
            nc=nc,
        )
        return tuple(outs)

    devices = jax.devices()[:n_cores]
    assert len(devices) == n_cores, (
        f"need {n_cores} neuron cores, have {len(jax.devices())}"
    )
    mesh = Mesh(np.asarray(devices), ("core",))
    spec = PartitionSpec("core")
    fn = jax.jit(
        shard_map(
            _body,
            mesh=mesh,
            in_specs=(spec,) * (n_params + len(out_names)),
            out_specs=(spec,) * len(out_names),
            check_rep=False,
        ),
        keep_unused=True,
    )
    sharding = NamedSharding(mesh, spec)
    zero_args = [
        jax.device_put(np.concatenate([z] * n_cores, axis=0), sharding)
        for z in zero_outs
    ]

    def run(in_maps, device_args=None):
        if device_args is None:
            device_args = [
                jax.device_put(
                    np.concatenate(
                        [np.asarray(m[name]) for m in in_maps], axis=0
                    ),
                    sharding,
                )
                for name in in_names
            ]
        outs = fn(*device_args, *zero_args)
        jax.block_until_ready(outs)
        res = []
        for c in range(n_cores):
            d = {}
            for i, name in enumerate(out_names):
                arr = np.asarray(outs[i])
                per = arr.shape[0] // n_cores
                d[name] = arr[c * per:(c + 1) * per]
            res.append(d)
        return res

    run.in_names = in_names
    run.sharding = sharding
    run.fn = fn
    run.zero_args = zero_args
    return run, out_names


def make_in_maps(x, input_perm, W, b):
    """Host-side sharding / layout transforms -> per-core input dicts."""
    bf16 = mybir.dt.np(BF16)
    toks = np.asarray(x, dtype=np.float32).reshape(TOK, F)
    wT = np.ascontiguousarray(
        np.transpose(np.asarray(W, dtype=np.float32), (2, 0, 1))
    ).reshape(GS, G * GO).astype(bf16)
    bT = np.ascontiguousarray(np.asarray(b, dtype=np.float32).T)
    perm = np.asarray(input_perm).astype(np.int64)
    # idx table, stage-blocked: stage fs occupies columns [fs*FS/16,
    # (fs+1)*FS/16); within a stage, local index j sits at row j%16,
    # column j//16 (the gather's 16-partition wrap), replicated x8.
    p16 = perm.astype(np.int16).reshape(NF, FS // 16, 16)
    idx_w = np.concatenate([s.T for s in p16], axis=1)        # [16, F//16]
    idx_full = np.ascontiguousarray(np.tile(idx_w, (8, 1)))   # [128, F//16]

    in_maps = []
    for c in range(N_CORES):
        shard = toks[c * TPC:(c + 1) * TPC]                   # [TPC, F]
        xT = np.ascontiguousarray(shard.T).astype(bf16)       # [F, TPC] bf16
        in_maps.append({"xT": xT, "wT": wT, "bT": bT, "idx": idx_full})
    return in_maps


def assemble_output(results, dtype):
    """Per-core yT [F, TPC] -> full y [B, S, F]."""
    parts = [
        np.ascontiguousarray(results[c]["yT"].astype(np.float32).T)
        for c in range(N_CORES)
    ]
    y = np.concatenate(parts, axis=0).reshape(B, S, F)
    return y.astype(dtype, copy=False)


_RUNNER_CACHE = {}


def _get_runner():
    if "run" not in _RUNNER_CACHE:
        nc = build_nc(reps=1)
        run, out_names = make_runner(nc)
        _RUNNER_CACHE["run"] = run
    return _RUNNER_CACHE["run"]


def kernel(**inputs) -> np.ndarray:
    x = inputs["x"]
    run = _get_runner()
    in_maps = make_in_maps(x, inputs["input_perm"], inputs["W"], inputs["b"])
    results = run(in_maps)
    return assemble_output(results, np.asarray(x).dtype)


if __name__ == "__main__":
    rng = np.random.default_rng(0)
    x = rng.standard_normal((B, S, F), dtype=np.float32)
    perm = rng.permutation(F).astype(np.int64)
    W = (rng.standard_normal((G, GO, GS), dtype=np.float32) / np.sqrt(GS))
    b = rng.standard_normal((G, GO), dtype=np.float32) * 0.01
    y = kernel(x=x, input_perm=perm, W=W, b=b)
    ref = np.einsum("bsgi,goi->bsgo",
                    x[..., perm].reshape(B, S, G, GS), W) + b
    ref = ref.reshape(B, S, F)
    err = np.abs(y - ref).max() / np.abs(ref).max()
    print("self-check rel err:", err)
